# revision 13
# baseline (speedup 1.0000x reference)
"""GCN encoder kernel for Trainium2 (8 NeuronCores).

Strategy (graph/data parallel, per sharding hint):
  - Nodes sharded by destination range across 8 cores (2500 -> padded 2560/core).
  - Host precomputes GCN symmetric normalization and a per-destination-group
    schedule: edges sorted by dst group (128 dsts/group), padded to a uniform
    number of 128-edge chunks C.  The segment-sum becomes dense matmuls with
    tiny scatter matrices S[edge_slot, dst] = norm.
  - Per layer: AllGather z (bf16) across cores, dma_gather the per-edge source
    rows, TensorE matmuls accumulate messages into PSUM per dst group (with the
    per-layer bias folded in as a rank-1 matmul), then GELU + LayerNorm +
    residual blend on Vector/Scalar engines.
"""

import sys
from dataclasses import dataclass

sys.path.insert(0, "/opt/trn_rl_repo")

import numpy as np
import ml_dtypes

import concourse.bass as bass
import concourse.tile as tile
from concourse import bacc, mybir
from concourse import bass_utils
from concourse.masks import make_identity

BF16 = ml_dtypes.bfloat16
ALPHA = 0.1
EPS = 1e-5
IN_DIM = 128
H = 256
NCORES = 8
ACT_FN = mybir.ActivationFunctionType.Gelu_apprx_tanh
SUB_IDX = 1024          # max indices per dma_gather instruction
SINGLE_PACKET = True


@dataclass(frozen=True)
class Cfg:
    n: int = 20000
    layers: int = 6

    @property
    def percore(self):
        return self.n // NCORES

    @property
    def groups(self):
        return (self.percore + 127) // 128

    @property
    def padn(self):
        return self.groups * 128


DEFAULT_CFG = Cfg()
_cache = {}


def _preprocess(cfg, x, edge_index, W_in, b_in, g_in, beta_in, Wc, bc, gc,
                betac):
    """Host-side graph preprocessing -> per-core input maps."""
    N, G, PERCORE, PADN, L = (cfg.n, cfg.groups, cfg.percore, cfg.padn,
                              cfg.layers)
    x = np.asarray(x, dtype=np.float32)
    ei = np.asarray(edge_index).astype(np.int64)
    W_in = np.asarray(W_in, dtype=np.float32)
    b_in = np.asarray(b_in, dtype=np.float32)
    g_in = np.asarray(g_in, dtype=np.float32)
    beta_in = np.asarray(beta_in, dtype=np.float32)
    Wc = np.asarray(Wc, dtype=np.float32)
    bc = np.asarray(bc, dtype=np.float32)
    gc = np.asarray(gc, dtype=np.float32)
    betac = np.asarray(betac, dtype=np.float32)

    loop = np.arange(N, dtype=np.int64)
    row = np.concatenate([ei[0], loop])   # src
    col = np.concatenate([ei[1], loop])   # dst
    deg = np.bincount(col, minlength=N).astype(np.float32)
    dinv = np.where(deg > 0, 1.0 / np.sqrt(deg), 0.0).astype(np.float32)
    norm = (dinv[row] * dinv[col]).astype(np.float32)

    # padded global source id: node s lives at row PADN*(s//PERCORE) + s%PERCORE
    pad_src = (row // PERCORE) * PADN + (row % PERCORE)

    core_of = col // PERCORE
    dloc_all = col % PERCORE
    grp_all = dloc_all // 128
    dst_all = dloc_all % 128

    per_core = []
    maxE = 1
    for c in range(NCORES):
        m = core_of == c
        g_e = grp_all[m]
        d_e = dst_all[m]
        s_e = pad_src[m]
        n_e = norm[m]
        order = np.lexsort((s_e, g_e))
        g_e, d_e, s_e, n_e = g_e[order], d_e[order], s_e[order], n_e[order]
        counts = np.bincount(g_e, minlength=G)
        maxE = max(maxE, counts.max())
        per_core.append((g_e, d_e, s_e, n_e, counts))

    C = int((maxE + 127) // 128)
    SLOTS = C * 128
    S16 = SLOTS // 16

    in_maps = []
    for c in range(NCORES):
        g_e, d_e, s_e, n_e, counts = per_core[c]
        starts = np.zeros(G + 1, dtype=np.int64)
        np.cumsum(counts, out=starts[1:])
        slot = np.arange(len(g_e)) - starts[g_e]

        idx_l = np.zeros((G, SLOTS), dtype=np.int16)      # pad -> row 0
        S_l = np.zeros((G, SLOTS, 128), dtype=np.float32)
        idx_l[g_e, slot] = s_e.astype(np.int16)
        S_l[g_e, slot, d_e] = n_e

        # idx layout: logical slot i -> partition 16*stripe + i%16, col i//16
        idx_rs = idx_l.reshape(G, S16, 16).transpose(2, 0, 1)   # [16, G, S16]
        gidx = np.tile(idx_rs, (8, 1, 1)).astype(np.int16)      # [128, G, S16]

        # S layout: [G, 128(slot%128), C(chunk), 128(dst)]
        S_arr = S_l.reshape(G, C, 128, 128).transpose(0, 2, 1, 3).astype(BF16)

        xs = x[c * PERCORE:(c + 1) * PERCORE]
        xT = np.zeros((IN_DIM, PADN), dtype=BF16)
        xT[:, :PERCORE] = xs.T.astype(BF16)

        in_maps.append({
            "xT": np.ascontiguousarray(xT),
            "gidx": np.ascontiguousarray(gidx),
            "S": np.ascontiguousarray(S_arr),
        })

    Wc_bf = np.ascontiguousarray(Wc.reshape(L, 2, 128, H).astype(BF16))
    W_in_bf = W_in.astype(BF16)
    biases = np.concatenate([b_in[None, :], bc], axis=0).astype(BF16)
    ln = np.zeros((2 + 2 * L, H), dtype=np.float32)
    ln[0] = g_in
    ln[1] = beta_in
    ln[2:2 + L] = (1.0 - ALPHA) * gc
    ln[2 + L:2 + 2 * L] = (1.0 - ALPHA) * betac
    for m in in_maps:
        m["Wc"] = Wc_bf
        m["W_in"] = W_in_bf
        m["biases"] = biases
        m["ln"] = ln

    return in_maps, C


def _build(cfg, C):
    """Build the Bass program (shared by all 8 cores)."""
    G, PADN, L = cfg.groups, cfg.padn, cfg.layers
    SLOTS = C * 128
    S16 = SLOTS // 16
    f32 = mybir.dt.float32
    bf16 = mybir.dt.bfloat16

    nc = bacc.Bacc("TRN2", target_bir_lowering=False, debug=False,
                   num_devices=NCORES)

    xT_in = nc.dram_tensor("xT", [IN_DIM, PADN], bf16, kind="ExternalInput")
    gidx_in = nc.dram_tensor("gidx", [128, G, S16], mybir.dt.int16,
                             kind="ExternalInput")
    S_in = nc.dram_tensor("S", [G, 128, C, 128], bf16, kind="ExternalInput")
    Wc_in = nc.dram_tensor("Wc", [L, 2, 128, H], bf16, kind="ExternalInput")
    W_in_in = nc.dram_tensor("W_in", [IN_DIM, H], bf16, kind="ExternalInput")
    biases_in = nc.dram_tensor("biases", [L + 1, H], bf16,
                               kind="ExternalInput")
    ln_in = nc.dram_tensor("ln", [2 + 2 * L, H], f32, kind="ExternalInput")
    out_dram = nc.dram_tensor("out", [PADN, H], f32, kind="ExternalOutput")

    # static internal DRAM buffers for the z exchange
    zbounces = [nc.dram_tensor(f"zbounce{l}", [PADN, H], bf16,
                               kind="Internal") for l in range(L)]
    zfulls = [nc.dram_tensor(f"zfull{l}", [NCORES * PADN, H], bf16,
                             kind="Internal") for l in range(L)]

    def bcast128(ap_row):
        # [1, H] DRAM row -> partition-broadcast [128, H]
        return bass.AP(tensor=ap_row.tensor, offset=ap_row.offset,
                       ap=[[0, 128]] + list(ap_row.ap[1:]))

    with tile.TileContext(nc) as tc:
        with (
            tc.tile_pool(name="persist", bufs=1) as pp,
            tc.tile_pool(name="msgs_pool", bufs=3) as msgs_pool,
            tc.tile_pool(name="s_pool", bufs=3) as s_pool,
            tc.tile_pool(name="small", bufs=3) as small,
            tc.tile_pool(name="psum_a", bufs=2, space="PSUM") as psum_a,
            tc.tile_pool(name="psum_z", bufs=2, space="PSUM") as psum_z,
            tc.tile_pool(name="psum_t", bufs=2, space="PSUM") as psum_t,
        ):
            # ---------- persistent tiles ----------
            xcur = pp.tile([128, G, H], f32)
            y_all = pp.tile([128, G, H], f32)
            h0s = pp.tile([128, G, H], f32)
            mv_all = pp.tile([128, G, 2], f32)
            rstd_all = pp.tile([128, G], f32)
            gidx_sb = pp.tile([128, G, S16], mybir.dt.int16)
            Wc_sb = pp.tile([128, L * 2, H], bf16)
            W_in_sb = pp.tile([128, H], bf16)
            bias_sb = pp.tile([1, L + 1, H], bf16)
            ones_sb = pp.tile([1, 128], bf16)
            ln_sb = pp.tile([128, 2 + 2 * L, H], f32)
            ident = pp.tile([128, 128], f32)
            xT_sb = pp.tile([128, PADN], bf16)
            eps_sb = pp.tile([128, 1], f32)

            nc.vector.memset(eps_sb[:], EPS)
            nc.sync.dma_start(out=gidx_sb[:], in_=gidx_in.ap())
            for l in range(L):
                for kt in range(2):
                    nc.sync.dma_start(out=Wc_sb[:, l * 2 + kt, :],
                                      in_=Wc_in.ap()[l, kt])
            nc.sync.dma_start(out=W_in_sb[:], in_=W_in_in.ap())
            nc.sync.dma_start(out=bias_sb[:], in_=biases_in.ap()[None])
            nc.vector.memset(ones_sb[:], 1.0)
            for r in range(2 + 2 * L):
                nc.sync.dma_start(out=ln_sb[:, r, :],
                                  in_=bcast128(ln_in.ap()[r:r + 1, :]))
            make_identity(nc, ident[:])
            nc.sync.dma_start(out=xT_sb[:], in_=xT_in.ap())

            def apply_ln_blend(g, gi, bi, first):
                """Phase C for group g: t=(y-mu)*rstd*gamma+beta [+h0s,+xcur]."""
                t = small.tile([128, H], f32, name="t_ln")
                nc.vector.tensor_scalar_sub(
                    out=t[:], in0=y_all[:, g, :], scalar1=mv_all[:, g, 0:1])
                u = small.tile([128, H], f32, name="u_ln")
                nc.vector.scalar_tensor_tensor(
                    out=u[:], in0=t[:], scalar=rstd_all[:, g:g + 1],
                    in1=ln_sb[:, gi, :],
                    op0=mybir.AluOpType.mult, op1=mybir.AluOpType.mult)
                if first:
                    nc.vector.tensor_tensor(
                        out=xcur[:, g, :], in0=u[:], in1=ln_sb[:, bi, :],
                        op=mybir.AluOpType.add)
                    nc.vector.tensor_scalar_mul(
                        out=h0s[:, g, :], in0=xcur[:, g, :], scalar1=ALPHA)
                else:
                    v = small.tile([128, H], f32, name="v_ln")
                    nc.vector.tensor_tensor(
                        out=v[:], in0=u[:], in1=ln_sb[:, bi, :],
                        op=mybir.AluOpType.add)
                    w = small.tile([128, H], f32, name="w_ln")
                    nc.vector.tensor_tensor(
                        out=w[:], in0=v[:], in1=h0s[:, g, :],
                        op=mybir.AluOpType.add)
                    nc.vector.tensor_tensor(
                        out=xcur[:, g, :], in0=xcur[:, g, :], in1=w[:],
                        op=mybir.AluOpType.add)

            def z_matmul(g, l, zbounce):
                """Transpose xcur[g] and compute z = xcur @ Wc[l] -> zbounce."""
                tp = psum_t.tile([128, 2, 128], f32, name="tp")
                xcurT = small.tile([128, 2, 128], bf16, name="xcurT")
                for kt in range(2):
                    nc.tensor.transpose(
                        out=tp[:, kt, :],
                        in_=xcur[:, g, kt * 128:(kt + 1) * 128],
                        identity=ident[:])
                    nc.scalar.activation(
                        out=xcurT[:, kt, :], in_=tp[:, kt, :],
                        func=mybir.ActivationFunctionType.Copy)
                zp = psum_z.tile([128, H], f32, name="zp")
                for kt in range(2):
                    nc.tensor.matmul(
                        out=zp[:], lhsT=xcurT[:, kt, :],
                        rhs=Wc_sb[:, l * 2 + kt, :],
                        start=(kt == 0), stop=(kt == 1))
                z_sb = small.tile([128, H], bf16, name="z_sb")
                nc.scalar.activation(
                    out=z_sb[:], in_=zp[:],
                    func=mybir.ActivationFunctionType.Copy)
                nc.sync.dma_start(
                    out=zbounce.ap()[g * 128:(g + 1) * 128, :], in_=z_sb[:])

            def batched_rstd():
                nc.scalar.activation(
                    out=rstd_all[:], in_=mv_all[:, :, 1],
                    func=mybir.ActivationFunctionType.Sqrt, bias=eps_sb[:])
                nc.vector.reciprocal(out=rstd_all[:], in_=rstd_all[:])

            def gelu_stats(g, psum):
                nc.scalar.activation(
                    out=y_all[:, g, :], in_=psum[:], func=ACT_FN)
                stats = small.tile([128, 6], f32, name="bn_st")
                nc.vector.bn_stats(out=stats[:], in_=y_all[:, g, :])
                nc.vector.bn_aggr(out=mv_all[:, g, :], in_=stats[:])

            # ---------- input block ----------
            for g in range(G):
                hp = psum_a.tile([128, H], f32, name="agg")
                nc.tensor.matmul(out=hp[:],
                                 lhsT=xT_sb[:, g * 128:(g + 1) * 128],
                                 rhs=W_in_sb[:], start=True, stop=False)
                nc.tensor.matmul(out=hp[:], lhsT=ones_sb[:],
                                 rhs=bias_sb[:, 0, :], start=False, stop=True)
                gelu_stats(g, hp)
            batched_rstd()
            for g in range(G):
                apply_ln_blend(g, 0, 1, first=True)
                z_matmul(g, 0, zbounces[0])

            for l in range(L):
                nc.gpsimd.collective_compute(
                    "AllGather", mybir.AluOpType.bypass,
                    replica_groups=[list(range(NCORES))],
                    ins=[zbounces[l].ap()], outs=[zfulls[l].ap()])

                # phase A: gather + scatter-matmul + gelu + stats
                for g in range(G):
                    msgs = msgs_pool.tile([128, C, H], bf16, name="msgs")
                    for j0 in range(0, SLOTS, SUB_IDX):
                        j1 = min(j0 + SUB_IDX, SLOTS)
                        nc.gpsimd.dma_gather(
                            msgs[:, j0 // 128:j1 // 128, :], zfulls[l].ap(),
                            gidx_sb[:, g, j0 // 16:j1 // 16],
                            num_idxs=j1 - j0, num_idxs_reg=j1 - j0,
                            elem_size=H, single_packet=SINGLE_PACKET)
                    s_sb = s_pool.tile([128, C, 128], bf16, name="s_sb")
                    nc.sync.dma_start(out=s_sb[:], in_=S_in.ap()[g])
                    agg = psum_a.tile([128, H], f32, name="agg")
                    for c in range(C):
                        nc.tensor.matmul(
                            out=agg[:], lhsT=s_sb[:, c, :], rhs=msgs[:, c, :],
                            start=(c == 0), stop=False)
                    nc.tensor.matmul(
                        out=agg[:], lhsT=ones_sb[:], rhs=bias_sb[:, 1 + l, :],
                        start=False, stop=True)
                    gelu_stats(g, agg)
                batched_rstd()
                # phase C: LN apply + blend + next z
                for g in range(G):
                    apply_ln_blend(g, 2 + l, 2 + L + l, first=False)
                    if l < L - 1:
                        z_matmul(g, l + 1, zbounces[l + 1])
                    else:
                        o_sb = small.tile([128, H], f32, name="o_sb")
                        nc.vector.tensor_copy(out=o_sb[:], in_=xcur[:, g, :])
                        nc.sync.dma_start(
                            out=out_dram.ap()[g * 128:(g + 1) * 128, :],
                            in_=o_sb[:])

    nc.compile()
    return nc


def _get_program(cfg, C):
    key = (cfg, C)
    if key not in _cache:
        _cache[key] = _build(cfg, C)
    return _cache[key]


def run_sharded(inputs, trace=False, cfg=DEFAULT_CFG):
    in_maps, C = _preprocess(cfg, **inputs)
    nc = _get_program(cfg, C)
    res = bass_utils.run_bass_kernel_spmd(
        nc, in_maps, core_ids=list(range(NCORES)), trace=trace)
    out = np.empty((cfg.n, H), dtype=np.float32)
    for c in range(NCORES):
        out[c * cfg.percore:(c + 1) * cfg.percore] = \
            res.results[c]["out"][:cfg.percore]
    return out, res


def kernel(**inputs):
    out, _ = run_sharded(inputs, trace=False)
    return out


# revision 23
# speedup vs baseline: 1.1580x; 1.1580x over previous
"""GCN encoder kernel for Trainium2 (8 NeuronCores).

Strategy (graph/data parallel, per sharding hint):
  - Nodes sharded by destination range across 8 cores (2500 -> padded 2560/core).
  - Host precomputes GCN symmetric normalization and a per-destination-group
    schedule: non-self edges are deduplicated by source per 128-dst group and
    padded with trailing -1 (skipped by the gather ucode).  The segment-sum
    becomes dense matmuls with tiny scatter matrices S[src_slot, dst] holding
    the summed edge norms; self-loops are applied as a diagonal matmul against
    the locally-kept z.
  - Per layer: AllGather z (bf16) across cores, dma_gather the unique source
    rows, TensorE matmuls accumulate messages into PSUM per dst group (with the
    per-layer bias folded in as a rank-1 matmul), then GELU + LayerNorm
    (rsqrt via Newton on VectorE; no ACT table switches) + residual blend.
"""

import sys
from dataclasses import dataclass

sys.path.insert(0, "/opt/trn_rl_repo")

import numpy as np
import ml_dtypes

import concourse.bass as bass
import concourse.tile as tile
from concourse import bacc, mybir
from concourse import bass_utils
from concourse.masks import make_identity

BF16 = ml_dtypes.bfloat16
ALPHA = 0.1
EPS = 1e-5
IN_DIM = 128
H = 256
NCORES = 8
ACT_FN = mybir.ActivationFunctionType.Gelu_apprx_tanh
QUAKE_MAGIC = 0x5F3759DF


@dataclass(frozen=True)
class Cfg:
    n: int = 20000
    layers: int = 6

    @property
    def percore(self):
        return self.n // NCORES

    @property
    def groups(self):
        return (self.percore + 127) // 128

    @property
    def padn(self):
        return self.groups * 128


DEFAULT_CFG = Cfg()
_cache = {}


def _preprocess(cfg, x, edge_index, W_in, b_in, g_in, beta_in, Wc, bc, gc,
                betac):
    """Host-side graph preprocessing -> per-core input maps."""
    N, G, PERCORE, PADN, L = (cfg.n, cfg.groups, cfg.percore, cfg.padn,
                              cfg.layers)
    x = np.asarray(x, dtype=np.float32)
    ei = np.asarray(edge_index).astype(np.int64)
    W_in = np.asarray(W_in, dtype=np.float32)
    b_in = np.asarray(b_in, dtype=np.float32)
    g_in = np.asarray(g_in, dtype=np.float32)
    beta_in = np.asarray(beta_in, dtype=np.float32)
    Wc = np.asarray(Wc, dtype=np.float32)
    bc = np.asarray(bc, dtype=np.float32)
    gc = np.asarray(gc, dtype=np.float32)
    betac = np.asarray(betac, dtype=np.float32)

    loop = np.arange(N, dtype=np.int64)
    col_all = np.concatenate([ei[1], loop])   # dst (for degree)
    deg = np.bincount(col_all, minlength=N).astype(np.float32)
    dinv = np.where(deg > 0, 1.0 / np.sqrt(deg), 0.0).astype(np.float32)

    # non-self edges (self loops handled by the diagonal matmul)
    row = ei[0]
    col = ei[1]
    norm = (dinv[row] * dinv[col]).astype(np.float32)
    pad_src = (row // PERCORE) * PADN + (row % PERCORE)

    core_of = col // PERCORE
    dloc_all = col % PERCORE
    grp_all = dloc_all // 128
    dst_all = dloc_all % 128

    per_core = []
    maxU = 1
    for c in range(NCORES):
        m = core_of == c
        g_e = grp_all[m]
        d_e = dst_all[m]
        s_e = pad_src[m]
        n_e = norm[m]
        # dedup sources within each group
        key = g_e * (NCORES * PADN) + s_e
        ukey, slot_of_edge = np.unique(key, return_inverse=True)
        u_grp = ukey // (NCORES * PADN)
        u_src = ukey % (NCORES * PADN)
        counts = np.bincount(u_grp, minlength=G)
        maxU = max(maxU, counts.max())
        per_core.append((g_e, d_e, s_e, n_e, slot_of_edge, u_grp, u_src,
                         counts))

    C = int((maxU + 127) // 128)
    SLOTS = C * 128
    S16 = SLOTS // 16

    in_maps = []
    for c in range(NCORES):
        g_e, d_e, s_e, n_e, slot_of_edge, u_grp, u_src, counts = per_core[c]
        starts = np.zeros(G + 1, dtype=np.int64)
        np.cumsum(counts, out=starts[1:])
        # slot of each unique (group, src) within its group
        u_slot = np.arange(len(u_grp)) - starts[u_grp]

        # Trailing -1s are trimmed by the gather ucode, but the decode-side
        # ring accounting uses num_idxs_reg rounded to 128-chunks — the
        # trimmed count must round to the same C chunks, so dummy-fill with
        # index 0 up to (C-1)*128+1 and use -1 only inside the last chunk.
        idx_l = np.full((G, SLOTS), -1, dtype=np.int16)
        idx_l[u_grp, u_slot] = u_src.astype(np.int16)
        min_cnt = (C - 1) * 128 + 1
        fill = (np.arange(SLOTS)[None, :] < min_cnt) & (idx_l < 0)
        idx_l[fill] = 0

        S_l = np.zeros((G, SLOTS, 128), dtype=np.float32)
        np.add.at(S_l, (g_e, u_slot[slot_of_edge], d_e), n_e)

        # idx layout: logical slot i -> partition 16*stripe + i%16, col i//16
        idx_rs = idx_l.reshape(G, S16, 16).transpose(2, 0, 1)   # [16, G, S16]
        gidx = np.tile(idx_rs, (8, 1, 1)).astype(np.int16)      # [128, G, S16]

        # S layout: [G, 128(slot%128), C(chunk), 128(dst)]
        S_arr = S_l.reshape(G, C, 128, 128).transpose(0, 2, 1, 3).astype(BF16)

        # self-loop diagonal: D[g, p, p] = dinv[node]^2
        nodes = np.arange(PERCORE) + c * PERCORE
        d2 = np.zeros(PADN, dtype=np.float32)
        d2[:PERCORE] = dinv[nodes] ** 2
        D_arr = np.zeros((G, 128, 128), dtype=np.float32)
        pi = np.arange(128)
        for g in range(G):
            D_arr[g, pi, pi] = d2[g * 128:(g + 1) * 128]
        D_arr = D_arr.astype(BF16)

        xs = x[c * PERCORE:(c + 1) * PERCORE]
        xT = np.zeros((IN_DIM, PADN), dtype=BF16)
        xT[:, :PERCORE] = xs.T.astype(BF16)

        in_maps.append({
            "xT": np.ascontiguousarray(xT),
            "gidx": np.ascontiguousarray(gidx),
            "S": np.ascontiguousarray(S_arr),
            "D": np.ascontiguousarray(D_arr),
        })

    Wc_bf = np.ascontiguousarray(Wc.reshape(L, 2, 128, H).astype(BF16))
    W_in_bf = W_in.astype(BF16)
    biases = np.concatenate([b_in[None, :], bc], axis=0).astype(BF16)
    ln = np.zeros((2 + 2 * L, H), dtype=np.float32)
    ln[0] = g_in
    ln[1] = beta_in
    ln[2:2 + L] = (1.0 - ALPHA) * gc
    ln[2 + L:2 + 2 * L] = (1.0 - ALPHA) * betac
    for m in in_maps:
        m["Wc"] = Wc_bf
        m["W_in"] = W_in_bf
        m["biases"] = biases
        m["ln"] = ln

    return in_maps, C


def _build(cfg, C):
    """Build the Bass program (shared by all 8 cores)."""
    G, PADN, L = cfg.groups, cfg.padn, cfg.layers
    SLOTS = C * 128
    S16 = SLOTS // 16
    f32 = mybir.dt.float32
    i32 = mybir.dt.int32
    bf16 = mybir.dt.bfloat16
    Alu = mybir.AluOpType

    nc = bacc.Bacc("TRN2", target_bir_lowering=False, debug=False,
                   num_devices=NCORES)

    xT_in = nc.dram_tensor("xT", [IN_DIM, PADN], bf16, kind="ExternalInput")
    gidx_in = nc.dram_tensor("gidx", [128, G, S16], mybir.dt.int16,
                             kind="ExternalInput")
    S_in = nc.dram_tensor("S", [G, 128, C, 128], bf16, kind="ExternalInput")
    D_in = nc.dram_tensor("D", [G, 128, 128], bf16, kind="ExternalInput")
    Wc_in = nc.dram_tensor("Wc", [L, 2, 128, H], bf16, kind="ExternalInput")
    W_in_in = nc.dram_tensor("W_in", [IN_DIM, H], bf16, kind="ExternalInput")
    biases_in = nc.dram_tensor("biases", [L + 1, H], bf16,
                               kind="ExternalInput")
    ln_in = nc.dram_tensor("ln", [2 + 2 * L, H], f32, kind="ExternalInput")
    out_dram = nc.dram_tensor("out", [PADN, H], f32, kind="ExternalOutput")

    zbounces = [nc.dram_tensor(f"zbounce{l}", [PADN, H], bf16,
                               kind="Internal") for l in range(L)]
    zfulls = [nc.dram_tensor(f"zfull{l}", [NCORES * PADN, H], bf16,
                             kind="Internal") for l in range(L)]

    def bcast128(ap_row):
        return bass.AP(tensor=ap_row.tensor, offset=ap_row.offset,
                       ap=[[0, 128]] + list(ap_row.ap[1:]))

    with tile.TileContext(nc) as tc:
        with (
            tc.tile_pool(name="persist", bufs=1) as pp,
            tc.tile_pool(name="msgs_pool", bufs=4) as msgs_pool,
            tc.tile_pool(name="s_pool", bufs=4) as s_pool,
            tc.tile_pool(name="small", bufs=4) as small,
            tc.tile_pool(name="tiny", bufs=6) as tiny,
            tc.tile_pool(name="psum_a", bufs=2, space="PSUM") as psum_a,
            tc.tile_pool(name="psum_z", bufs=2, space="PSUM") as psum_z,
            tc.tile_pool(name="psum_t", bufs=2, space="PSUM") as psum_t,
        ):
            # ---------- persistent tiles ----------
            xcur = pp.tile([128, G, H], f32)
            h0s = pp.tile([128, G, H], f32)
            z_all = pp.tile([128, G, H], bf16)
            msgs_all = pp.tile([128, 4, C, H], bf16)
            gidx_sb = pp.tile([128, G, S16], mybir.dt.int16)
            D_sb = pp.tile([128, G, 128], bf16)
            Wc_sb = pp.tile([128, L * 2, H], bf16)
            W_in_sb = pp.tile([128, H], bf16)
            bias_sb = pp.tile([1, L + 1, H], bf16)
            ones_sb = pp.tile([1, 128], bf16)
            ln_sb = pp.tile([128, 2 + 2 * L, H], f32)
            ident = pp.tile([128, 128], f32)
            xT_sb = pp.tile([128, PADN], bf16)

            nc.sync.dma_start(out=gidx_sb[:], in_=gidx_in.ap())
            for g in range(G):
                nc.sync.dma_start(out=D_sb[:, g, :], in_=D_in.ap()[g])
            for l in range(L):
                for kt in range(2):
                    nc.sync.dma_start(out=Wc_sb[:, l * 2 + kt, :],
                                      in_=Wc_in.ap()[l, kt])
            nc.sync.dma_start(out=W_in_sb[:], in_=W_in_in.ap())
            nc.sync.dma_start(out=bias_sb[:], in_=biases_in.ap()[None])
            nc.vector.memset(ones_sb[:], 1.0)
            for r in range(2 + 2 * L):
                nc.sync.dma_start(out=ln_sb[:, r, :],
                                  in_=bcast128(ln_in.ap()[r:r + 1, :]))
            make_identity(nc, ident[:])
            nc.sync.dma_start(out=xT_sb[:], in_=xT_in.ap())

            def quake_rstd(var_ap):
                """rstd = 1/sqrt(var+eps) on DVE only (Newton, 2 iters)."""
                v = tiny.tile([128, 1], f32, name="q_v")
                h = tiny.tile([128, 1], f32, name="q_h")
                r = tiny.tile([128, 1], f32, name="q_r")
                a = tiny.tile([128, 1], f32, name="q_a")
                nc.vector.tensor_scalar_add(out=v[:], in0=var_ap,
                                            scalar1=float(EPS))
                nc.vector.tensor_scalar_mul(out=h[:], in0=v[:], scalar1=0.5)
                vi = v[:].bitcast(i32)
                ri = r[:].bitcast(i32)
                nc.vector.tensor_scalar(out=ri, in0=vi, scalar1=1,
                                        scalar2=None,
                                        op0=Alu.logical_shift_right)
                nc.vector.tensor_scalar(out=ri, in0=ri, scalar1=-1,
                                        scalar2=QUAKE_MAGIC, op0=Alu.mult,
                                        op1=Alu.add)
                for _ in range(2):
                    nc.vector.tensor_tensor(out=a[:], in0=r[:], in1=r[:],
                                            op=Alu.mult)
                    nc.vector.tensor_tensor(out=a[:], in0=a[:], in1=h[:],
                                            op=Alu.mult)
                    nc.vector.tensor_scalar(out=a[:], in0=a[:], scalar1=-1.0,
                                            scalar2=1.5, op0=Alu.mult,
                                            op1=Alu.add)
                    nc.vector.tensor_tensor(out=r[:], in0=r[:], in1=a[:],
                                            op=Alu.mult)
                return r

            def ln_blend_z(g, psum, gi, bi, l, first):
                """GELU + LN + blend for group g, then z for layer l."""
                y = small.tile([128, H], f32, name="y_g")
                nc.scalar.activation(out=y[:], in_=psum[:], func=ACT_FN)
                stats = tiny.tile([128, 6], f32, name="bn_st")
                nc.vector.bn_stats(out=stats[:], in_=y[:])
                mv = tiny.tile([128, 2], f32, name="bn_mv")
                nc.vector.bn_aggr(out=mv[:], in_=stats[:])
                rstd = quake_rstd(mv[:, 1:2])
                t = small.tile([128, H], f32, name="t_ln")
                nc.vector.tensor_scalar_sub(out=t[:], in0=y[:],
                                            scalar1=mv[:, 0:1])
                u = small.tile([128, H], f32, name="u_ln")
                nc.vector.scalar_tensor_tensor(
                    out=u[:], in0=t[:], scalar=rstd[:], in1=ln_sb[:, gi, :],
                    op0=Alu.mult, op1=Alu.mult)
                if first:
                    nc.vector.tensor_tensor(out=xcur[:, g, :], in0=u[:],
                                            in1=ln_sb[:, bi, :], op=Alu.add)
                    nc.vector.tensor_scalar_mul(out=h0s[:, g, :],
                                                in0=xcur[:, g, :],
                                                scalar1=ALPHA)
                else:
                    v = small.tile([128, H], f32, name="v_ln")
                    nc.vector.tensor_tensor(out=v[:], in0=u[:],
                                            in1=ln_sb[:, bi, :], op=Alu.add)
                    w = small.tile([128, H], f32, name="w_ln")
                    nc.vector.tensor_tensor(out=w[:], in0=v[:],
                                            in1=h0s[:, g, :], op=Alu.add)
                    nc.vector.tensor_tensor(out=xcur[:, g, :],
                                            in0=xcur[:, g, :], in1=w[:],
                                            op=Alu.add)
                if l is not None:
                    # transpose xcur[g], z = xcur @ Wc[l] -> z_all + zbounce
                    tp = psum_t.tile([128, 2, 128], f32, name="tp")
                    xcurT = small.tile([128, 2, 128], bf16, name="xcurT")
                    for kt in range(2):
                        nc.tensor.transpose(
                            out=tp[:, kt, :],
                            in_=xcur[:, g, kt * 128:(kt + 1) * 128],
                            identity=ident[:])
                        nc.scalar.activation(
                            out=xcurT[:, kt, :], in_=tp[:, kt, :],
                            func=mybir.ActivationFunctionType.Copy)
                    zp = psum_z.tile([128, H], f32, name="zp")
                    for kt in range(2):
                        nc.tensor.matmul(
                            out=zp[:], lhsT=xcurT[:, kt, :],
                            rhs=Wc_sb[:, l * 2 + kt, :],
                            start=(kt == 0), stop=(kt == 1))
                    nc.scalar.activation(
                        out=z_all[:, g, :], in_=zp[:],
                        func=mybir.ActivationFunctionType.Copy)
                    nc.sync.dma_start(
                        out=zbounces[l].ap()[g * 128:(g + 1) * 128, :],
                        in_=z_all[:, g, :])
                else:
                    nc.sync.dma_start(
                        out=out_dram.ap()[g * 128:(g + 1) * 128, :],
                        in_=xcur[:, g, :])

            # clear msgs buffers once: -1 pad slots are never written by the
            # gather, and S weights of 0 must multiply finite values
            nc.vector.memset(msgs_all[:], 0.0)

            # ---------- input block ----------
            for g in range(G):
                hp = psum_a.tile([128, H], f32, name="agg")
                nc.tensor.matmul(out=hp[:],
                                 lhsT=xT_sb[:, g * 128:(g + 1) * 128],
                                 rhs=W_in_sb[:], start=True, stop=False)
                nc.tensor.matmul(out=hp[:], lhsT=ones_sb[:],
                                 rhs=bias_sb[:, 0, :], start=False, stop=True)
                ln_blend_z(g, hp, 0, 1, 0, first=True)

            for l in range(L):
                nc.gpsimd.collective_compute(
                    "AllGather", mybir.AluOpType.bypass,
                    replica_groups=[list(range(NCORES))],
                    ins=[zbounces[l].ap()], outs=[zfulls[l].ap()])

                for g in range(G):
                    msgs = msgs_all[:, (l * G + g) % 4, :, :]
                    nc.gpsimd.dma_gather(
                        msgs, zfulls[l].ap(), gidx_sb[:, g, :],
                        num_idxs=SLOTS, num_idxs_reg=SLOTS, elem_size=H,
                        single_packet=False)
                    s_sb = s_pool.tile([128, C, 128], bf16, name="s_sb")
                    nc.sync.dma_start(out=s_sb[:], in_=S_in.ap()[g])
                    agg = psum_a.tile([128, H], f32, name="agg")
                    # self-loop diagonal first
                    nc.tensor.matmul(out=agg[:], lhsT=D_sb[:, g, :],
                                     rhs=z_all[:, g, :], start=True,
                                     stop=False)
                    for c in range(C):
                        nc.tensor.matmul(
                            out=agg[:], lhsT=s_sb[:, c, :], rhs=msgs[:, c, :],
                            start=False, stop=False)
                    del msgs
                    nc.tensor.matmul(
                        out=agg[:], lhsT=ones_sb[:], rhs=bias_sb[:, 1 + l, :],
                        start=False, stop=True)
                    ln_blend_z(g, agg, 2 + l, 2 + L + l,
                               l + 1 if l < L - 1 else None, first=False)

    nc.compile()
    return nc


def _get_program(cfg, C):
    key = (cfg, C)
    if key not in _cache:
        _cache[key] = _build(cfg, C)
    return _cache[key]


def run_sharded(inputs, trace=False, cfg=DEFAULT_CFG):
    in_maps, C = _preprocess(cfg, **inputs)
    nc = _get_program(cfg, C)
    res = bass_utils.run_bass_kernel_spmd(
        nc, in_maps, core_ids=list(range(NCORES)), trace=trace)
    out = np.empty((cfg.n, H), dtype=np.float32)
    for c in range(NCORES):
        out[c * cfg.percore:(c + 1) * cfg.percore] = \
            res.results[c]["out"][:cfg.percore]
    return out, res


def kernel(**inputs):
    out, _ = run_sharded(inputs, trace=False)
    return out
